# revision 1
# baseline (speedup 1.0000x reference)
"""AttentionFusionBlock Trainium2 kernel (8 NeuronCores, SPMD data-parallel).

Problem: B=2, C=256, H=W=64 (N=4096 tokens), 8 heads x d=32, attention +
residual + MLP(4C) fused block.

Sharding: core i owns batch b=i//4 and query-token quarter q=(i%4)*1024.
Each core computes K/V projections for the full 4096 tokens of its batch
(duplicated work across the 4 cores of a batch group, ~15us) which avoids
all all-reduces entirely. Output is channel-major [256, 1024] per core and
is reassembled on host.
"""

import numpy as np
import ml_dtypes

import concourse.bass as bass
import concourse.tile as tile
from concourse import bacc, mybir
from concourse import bass_utils

F32 = mybir.dt.float32
BF16 = mybir.dt.bfloat16
AF = mybir.ActivationFunctionType

C = 256          # d_model
NH = 8           # heads
D = 32           # head dim
N = 4096         # tokens per batch (64*64)
NQ = 1024        # query tokens per core
KT = 32          # number of 128-wide k tiles
SCALE = float(D) ** -0.5

_CACHE = {}


def _build(reps=1):
    nc = bacc.Bacc("TRN2", target_bir_lowering=False, debug=False, num_devices=8)

    # ---- DRAM I/O ----------------------------------------------------------
    xq = nc.dram_tensor("xq", [2, 128, NQ], F32, kind="ExternalInput").ap()
    xl = nc.dram_tensor("xl", [2, 128, N], BF16, kind="ExternalInput").ap()
    wqT = nc.dram_tensor("wqT", [2, 128, C], BF16, kind="ExternalInput").ap()
    wkT = nc.dram_tensor("wkT", [2, 128, C], BF16, kind="ExternalInput").ap()
    wvT = nc.dram_tensor("wvT", [2, 128, C], BF16, kind="ExternalInput").ap()
    woT = nc.dram_tensor("woT", [2, 128, C], BF16, kind="ExternalInput").ap()
    w1T = nc.dram_tensor("w1T", [2, 128, 1024], BF16, kind="ExternalInput").ap()
    w2T = nc.dram_tensor("w2T", [8, 128, C], BF16, kind="ExternalInput").ap()
    bqv = nc.dram_tensor("bqv", [3, 128, 1], F32, kind="ExternalInput").ap()
    bkv = nc.dram_tensor("bkv", [3, 128, 1], F32, kind="ExternalInput").ap()
    bvv = nc.dram_tensor("bvv", [1, C], F32, kind="ExternalInput").ap()
    bov = nc.dram_tensor("bov", [2, 128, 1], F32, kind="ExternalInput").ap()
    b1v = nc.dram_tensor("b1v", [8, 128, 1], F32, kind="ExternalInput").ap()
    b2v = nc.dram_tensor("b2v", [2, 128, 1], F32, kind="ExternalInput").ap()
    blk1v = nc.dram_tensor("blk1v", [1, 32], F32, kind="ExternalInput").ap()
    out = nc.dram_tensor("out", [2, 128, NQ], F32, kind="ExternalOutput").ap()

    with tile.TileContext(nc) as tc:
        for _ in range(reps):
            _body(tc, xq, xl, wqT, wkT, wvT, woT, w1T, w2T,
                  bqv, bkv, bvv, bov, b1v, b2v, blk1v, out)

    nc.compile()
    return nc


def _body(tc, xq, xl, wqT, wkT, wvT, woT, w1T, w2T,
          bqv, bkv, bvv, bov, b1v, b2v, blk1v, out):
    nc = tc.nc
    from contextlib import ExitStack

    ctx = ExitStack()
    with ctx:
        singles = ctx.enter_context(tc.tile_pool(name="singles", bufs=1))

        # ---- load inputs/weights to SBUF -----------------------------------
        xl_s = [singles.tile([128, N], BF16, tag=f"xl{i}", name=f"xl{i}") for i in range(2)]
        xq_s = [singles.tile([128, NQ], F32, tag=f"xq{i}", name=f"xq{i}") for i in range(2)]
        wq_s = [singles.tile([128, C], BF16, tag=f"wq{i}", name=f"wq{i}") for i in range(2)]
        wk_s = [singles.tile([128, C], BF16, tag=f"wk{i}", name=f"wk{i}") for i in range(2)]
        wv_s = [singles.tile([128, C], BF16, tag=f"wv{i}", name=f"wv{i}") for i in range(2)]
        wo_s = [singles.tile([128, C], BF16, tag=f"wo{i}", name=f"wo{i}") for i in range(2)]
        w1_s = [singles.tile([128, 1024], BF16, tag=f"w1{i}", name=f"w1{i}") for i in range(2)]
        w2_s = [singles.tile([128, C], BF16, tag=f"w2{i}", name=f"w2{i}") for i in range(8)]
        bq_s = [singles.tile([128, 1], F32, tag=f"bq{i}", name=f"bq{i}") for i in range(3)]
        bk_s = [singles.tile([128, 1], F32, tag=f"bk{i}", name=f"bk{i}") for i in range(3)]
        bo_s = [singles.tile([128, 1], F32, tag=f"bo{i}", name=f"bo{i}") for i in range(2)]
        b1_s = [singles.tile([128, 1], F32, tag=f"b1{i}", name=f"b1{i}") for i in range(8)]
        b2_s = [singles.tile([128, 1], F32, tag=f"b2{i}", name=f"b2{i}") for i in range(2)]
        # bv is a free-dim bias -> DMA-replicate across all 128 partitions
        bvb_s = singles.tile([128, C], F32, tag="bvb")

        for i in range(2):
            nc.sync.dma_start(xl_s[i][:], xl[i])
            nc.sync.dma_start(xq_s[i][:], xq[i])
            nc.sync.dma_start(wq_s[i][:], wqT[i])
            nc.sync.dma_start(wk_s[i][:], wkT[i])
            nc.sync.dma_start(wv_s[i][:], wvT[i])
            nc.sync.dma_start(wo_s[i][:], woT[i])
            nc.sync.dma_start(w1_s[i][:], w1T[i])
            nc.sync.dma_start(bo_s[i][:], bov[i])
            nc.sync.dma_start(b2_s[i][:], b2v[i])
        for i in range(8):
            nc.sync.dma_start(w2_s[i][:], w2T[i])
            nc.sync.dma_start(b1_s[i][:], b1v[i])
        for i in range(3):
            nc.sync.dma_start(bq_s[i][:], bqv[i])
            nc.sync.dma_start(bk_s[i][:], bkv[i])
        bv_bcast = bass.AP(tensor=bvv.tensor, offset=bvv.offset,
                           ap=[[0, 128], [1, C]])
        nc.sync.dma_start(bvb_s[:], bv_bcast)

        # bf16 copy of xq for the Q projection rhs
        xqb_s = [singles.tile([128, NQ], BF16, tag=f"xqb{i}", name=f"xqb{i}") for i in range(2)]
        for i in range(2):
            nc.vector.tensor_copy(xqb_s[i][:], xq_s[i][:])

        # ---- projections ----------------------------------------------------
        kT_s = [singles.tile([128, N], BF16, tag=f"kT{i}", name=f"kT{i}") for i in range(3)]
        qT_s = [singles.tile([128, NQ], BF16, tag=f"qT{i}", name=f"qT{i}") for i in range(3)]
        HSL = [(0, 96), (96, 192), (192, 256)]  # channel range per kT/qT tile
        # V' layout: [128 k-part, KT * (8 heads * 33)]; col 33h+32 is the ones
        # column that yields the softmax row-sum during the PV matmul.
        v_s = singles.tile([128, KT * 264], BF16, tag="v")
        ones_ap = v_s[:].rearrange("p (t g c) -> p t g c", t=KT, c=33)[:, :, :, 32:33]
        nc.vector.memset(ones_ap, 1.0)

        with tc.tile_pool(name="ppsum", bufs=4, space="PSUM") as pp:
            # K^T = Wk @ Xl^T  (channel-major, bias per partition)
            for ti, (lo, hi) in enumerate(HSL):
                sz = hi - lo
                for t8 in range(8):
                    ps = pp.tile([128, 512], F32, tag="proj", name="proj_ps")
                    for ci in range(2):
                        nc.tensor.matmul(
                            ps[0:sz, :], wk_s[ci][:, lo:hi],
                            xl_s[ci][:, t8 * 512:(t8 + 1) * 512],
                            start=(ci == 0), stop=(ci == 1))
                    nc.vector.tensor_scalar_add(
                        kT_s[ti][0:sz, t8 * 512:(t8 + 1) * 512], ps[0:sz, :],
                        bk_s[ti][0:sz, :])
            # Q^T = Wq @ Xq^T
            for ti, (lo, hi) in enumerate(HSL):
                sz = hi - lo
                for t8 in range(2):
                    ps = pp.tile([128, 512], F32, tag="proj", name="proj_ps")
                    for ci in range(2):
                        nc.tensor.matmul(
                            ps[0:sz, :], wq_s[ci][:, lo:hi],
                            xqb_s[ci][:, t8 * 512:(t8 + 1) * 512],
                            start=(ci == 0), stop=(ci == 1))
                    nc.vector.tensor_scalar_add(
                        qT_s[ti][0:sz, t8 * 512:(t8 + 1) * 512], ps[0:sz, :],
                        bq_s[ti][0:sz, :])
            # V token-major: V[k_tile, c] = Xl_tile^T.T @ WvT ; bias along free
            for kt in range(KT):
                ps = pp.tile([128, 256], F32, tag="projv", name="projv_ps")
                for ci in range(2):
                    nc.tensor.matmul(
                        ps[:], xl_s[ci][:, kt * 128:(kt + 1) * 128],
                        wv_s[ci][:, 0:C],
                        start=(ci == 0), stop=(ci == 1))
                dst = v_s[:].rearrange("p (t g c) -> p t g c", t=KT, c=33)[
                    :, kt, :, 0:32]
                src = ps[:].rearrange("p (g c) -> p g c", c=32)
                nc.vector.tensor_tensor(
                    dst, src,
                    bvb_s[:].rearrange("p (g c) -> p g c", c=32),
                    mybir.AluOpType.add)

        # ---- attention ------------------------------------------------------
        attT_s = [singles.tile([128, NQ], BF16, tag=f"attT{i}", name=f"attT{i}") for i in range(2)]
        # block-ones for rowsum broadcast: [2, 64] with ones at [a, 32a:32a+32]
        blk1_s = singles.tile([1, 32], F32, tag="blk1")
        nc.sync.dma_start(blk1_s[:], blk1v[:])

        with tc.tile_pool(name="spsum", bufs=2, space="PSUM") as sp_pool, \
             tc.tile_pool(name="pvpsum", bufs=2, space="PSUM") as pv_pool, \
             tc.tile_pool(name="ptile", bufs=3) as pt_pool, \
             tc.tile_pool(name="norm", bufs=2) as norm_pool:
            pv_tiles = {}
            for h in range(NH):
                ch, r = h // 3, 32 * (h % 3)
                pv = pv_pool.tile([33, NQ], F32, tag="pv", name="pv_ps")
                pv_tiles[h] = pv
                for kt in range(KT):
                    sp = sp_pool.tile([128, NQ], F32, tag="s", name="s_ps")
                    for qh in range(2):
                        nc.tensor.matmul(
                            sp[:, qh * 512:(qh + 1) * 512],
                            kT_s[ch][r:r + 32, kt * 128:(kt + 1) * 128],
                            qT_s[ch][r:r + 32, qh * 512:(qh + 1) * 512],
                            start=True, stop=True)
                    pT = pt_pool.tile([128, NQ], BF16, tag="pT", name="pT_t")
                    nc.scalar.activation(pT[:], sp[:], AF.Exp, scale=SCALE)
                    voff = kt * 264 + 33 * h
                    for qh in range(2):
                        nc.tensor.matmul(
                            pv[:, qh * 512:(qh + 1) * 512],
                            v_s[:, voff:voff + 33],
                            pT[:, qh * 512:(qh + 1) * 512],
                            start=(kt == 0), stop=(kt == KT - 1))
                # normalize head h: reciprocal rowsum, broadcast via PE,
                # multiply on DVE
                rsi = norm_pool.tile([1, NQ], F32, tag="rs1", name="rs1_t")
                nc.vector.reciprocal(rsi[:], pv[32:33, :])
                bc = sp_pool.tile([32, NQ], F32, tag="s", name="bc_ps")
                for qh in range(2):
                    nc.tensor.matmul(
                        bc[:, qh * 512:(qh + 1) * 512], blk1_s[:],
                        rsi[:, qh * 512:(qh + 1) * 512],
                        start=True, stop=True)
                bcs = norm_pool.tile([32, NQ], F32, tag="bcs", name="bcs_t")
                nc.vector.tensor_copy(bcs[:], bc[:])
                cch, rr = h // 4, 32 * (h % 4)
                nc.vector.tensor_tensor(
                    attT_s[cch][rr:rr + 32, :],
                    pv_tiles[h][0:32, :],
                    bcs[:],
                    mybir.AluOpType.mult)
                del pv_tiles[h]

        # ---- out projection + residual --------------------------------------
        t_f = [singles.tile([128, NQ], F32, tag=f"tf{i}", name=f"tf{i}") for i in range(2)]
        t_b = [singles.tile([128, NQ], BF16, tag=f"tb{i}", name=f"tb{i}") for i in range(2)]
        with tc.tile_pool(name="opsum", bufs=4, space="PSUM") as op_pool, \
             tc.tile_pool(name="ostage", bufs=3) as os_pool:
            for co in range(2):
                for qh in range(2):
                    ps = op_pool.tile([128, 512], F32, tag="o", name="o_ps")
                    for ci in range(2):
                        nc.tensor.matmul(
                            ps[:], wo_s[ci][:, co * 128:(co + 1) * 128],
                            attT_s[ci][:, qh * 512:(qh + 1) * 512],
                            start=(ci == 0), stop=(ci == 1))
                    sl = slice(qh * 512, (qh + 1) * 512)
                    nc.vector.scalar_tensor_tensor(
                        t_f[co][:, sl], ps[:], bo_s[co][:], xq_s[co][:, sl],
                        mybir.AluOpType.add, mybir.AluOpType.add)
                nc.vector.tensor_copy(t_b[co][:], t_f[co][:])

            # ---- MLP --------------------------------------------------------
            hdn_s = [singles.tile([128, NQ], BF16, tag=f"hdn{i}", name=f"hdn{i}")
                     for i in range(8)]
            for hc in range(8):
                for qh in range(2):
                    ps = op_pool.tile([128, 512], F32, tag="o", name="o_ps")
                    for ci in range(2):
                        nc.tensor.matmul(
                            ps[:], w1_s[ci][:, hc * 128:(hc + 1) * 128],
                            t_b[ci][:, qh * 512:(qh + 1) * 512],
                            start=(ci == 0), stop=(ci == 1))
                    nc.scalar.activation(
                        hdn_s[hc][:, qh * 512:(qh + 1) * 512], ps[:],
                        AF.Gelu, bias=b1_s[hc][:], scale=1.0)
            for co in range(2):
                for qh in range(2):
                    ps = op_pool.tile([128, 512], F32, tag="o", name="o_ps")
                    for hc in range(8):
                        nc.tensor.matmul(
                            ps[:], w2_s[hc][:, co * 128:(co + 1) * 128],
                            hdn_s[hc][:, qh * 512:(qh + 1) * 512],
                            start=(hc == 0), stop=(hc == 7))
                    sl = slice(qh * 512, (qh + 1) * 512)
                    ot = os_pool.tile([128, 512], F32, tag="ot", name="ot_t")
                    nc.vector.scalar_tensor_tensor(
                        ot[:], ps[:], b2_s[co][:], t_f[co][:, sl],
                        mybir.AluOpType.add, mybir.AluOpType.add)
                    nc.sync.dma_start(out[co][:, sl], ot[:])


def _get_graph(reps=1):
    key = f"nc{reps}"
    if key not in _CACHE:
        _CACHE[key] = _build(reps)
    return _CACHE[key]


def kernel(query_feat, lateral_feat, Wq, bq, Wk, bk, Wv, bv, Wo, bo,
           W1, b1, W2, b2):
    nc = _get_graph()
    B = query_feat.shape[0]
    bf = ml_dtypes.bfloat16

    qf = np.asarray(query_feat, np.float32).reshape(B, C, N)
    lf = np.asarray(lateral_feat, np.float32).reshape(B, C, N)

    def prep():
        d = {}
        d["wqT"] = np.ascontiguousarray(np.asarray(Wq, np.float32).T).astype(bf).reshape(2, 128, C)
        d["wkT"] = np.ascontiguousarray(np.asarray(Wk, np.float32).T).astype(bf).reshape(2, 128, C)
        d["wvT"] = np.ascontiguousarray(np.asarray(Wv, np.float32).T).astype(bf).reshape(2, 128, C)
        d["woT"] = np.ascontiguousarray(np.asarray(Wo, np.float32).T).astype(bf).reshape(2, 128, C)
        d["w1T"] = np.ascontiguousarray(np.asarray(W1, np.float32).T).astype(bf).reshape(2, 128, 1024)
        d["w2T"] = np.ascontiguousarray(np.asarray(W2, np.float32).T).astype(bf).reshape(8, 128, C)
        def pack3(b):
            b = np.asarray(b, np.float32)
            o = np.zeros((3, 128, 1), np.float32)
            o[0, 0:96, 0] = b[0:96]
            o[1, 0:96, 0] = b[96:192]
            o[2, 0:64, 0] = b[192:256]
            return o
        d["blk1v"] = np.ones((1, 32), np.float32)
        d["bqv"] = pack3(bq)
        d["bkv"] = pack3(bk)
        d["bvv"] = np.asarray(bv, np.float32).reshape(1, C)
        d["bov"] = np.asarray(bo, np.float32).reshape(2, 128, 1)
        d["b1v"] = np.asarray(b1, np.float32).reshape(8, 128, 1)
        d["b2v"] = np.asarray(b2, np.float32).reshape(2, 128, 1)
        return d

    shared = prep()
    in_maps = []
    for core in range(8):
        b, qs = core // 4, (core % 4) * NQ
        m = dict(shared)
        m["xq"] = np.ascontiguousarray(qf[b][:, qs:qs + NQ]).reshape(2, 128, NQ)
        m["xl"] = lf[b].astype(bf).reshape(2, 128, N)
        in_maps.append(m)

    _CACHE["last_in_maps"] = in_maps
    res = bass_utils.run_bass_kernel_spmd(nc, in_maps, core_ids=list(range(8)))

    full = np.empty((B, C, N), np.float32)
    for core in range(8):
        b, qs = core // 4, (core % 4) * NQ
        full[b][:, qs:qs + NQ] = res.results[core]["out"].reshape(C, NQ)
    return full.reshape(B, C, 64, 64)

